# revision 8
# baseline (speedup 1.0000x reference)
"""Trainium2 Bass kernel for nn_GroupDenseFull.

Math: z[b, t*8+v] = sum_{s,w} x[b, s*8+w] * kernel_seq[s,w,v] * kernel_full[s,t]

Two-step structure (7.5x fewer FLOPs than the fused x @ Wc):
  step 1 (grouped):  y[b,s,v] = sum_w x[b,s,w] * ks[s,w,v]
  step 2 (mixing):   z[b,t,v] = sum_s y[b,s,v] * kf[s,t]

Device-side design:
  - bf16 I/O. Host pre-transposes x to channel-major [8k, 128c, B] bf16 so the
    device never transposes; host reassembles the transposed bf16 z output.
  - step 1 on PE: per k-tile of 128 channels ((s,w) interleaved, 16 groups),
    stationary = 128x128 block-diagonal ks matrix -> yT tiles with (s,v)
    interleaved partitions. Full PE utilization, 1 cyc/row (bf16, 512-mov).
  - deinterleave (s,v)-interleaved yT -> v-separated y_v via SBUF->SBUF DMA:
    per (v,k) one DMA of 16 partition lines (stride 8 -> contiguous), 4KB
    contiguous per line.
  - step 2 on PE: stationary = kf, moving = y_v [s, b] -> zT_v [t, b].
  - PSUM evict-casts (f32 -> bf16) spread across ACT, DVE, and GpSimd.
"""

import os
from contextlib import ExitStack

import numpy as np
import ml_dtypes

import concourse.bass as bass
import concourse.tile as tile
from concourse import bacc, mybir
from concourse.bass_utils import run_bass_kernel_spmd

B, C, W, S = 131072, 1024, 8, 128
NCORES = 8
BSH = B // NCORES          # 16384 rows per core
NK = 8                     # channel k-tiles of 128
NV = 8                     # v planes
SC = 2048                  # batch super-chunk (4KB bf16 DMA lines)
NSC = BSH // SC            # 8 super-chunks
MOV = 512                  # matmul moving width
NJ = SC // MOV             # 4 moving blocks per super-chunk

F32 = mybir.dt.float32
BF16 = mybir.dt.bfloat16

TRACE = bool(int(os.environ.get("KERNEL_TRACE", "0")))
LAST_EXEC_NS = None
LAST_TRACE_DIR = None

_cache = {}


def _setup_trace_shim():
    import sys
    import types

    import antenv
    from trn_agent_boot.trn_boot import _ntff_profile_via_ctypes

    if "antenv.axon_hooks" in sys.modules:
        return
    mod = types.ModuleType("antenv.axon_hooks")
    mod._hook = _ntff_profile_via_ctypes("/opt/axon/libaxon_pjrt.so")
    mod.get_axon_ntff_profile_hook = lambda: mod._hook
    mod.set_axon_ntff_profile_hook = lambda h: setattr(mod, "_hook", h)
    sys.modules["antenv.axon_hooks"] = mod
    antenv.axon_hooks = mod
    import concourse.bass_utils as bu

    bu.upload_artifacts = lambda tmpdir: tmpdir


def _evict_op(nc, idx, out, in_):
    """Spread PSUM->SBUF evict-casts across ACT / DVE / Pool."""
    m = idx % 8
    if m < 3:
        nc.scalar.copy(out=out, in_=in_)
    else:
        nc.vector.tensor_copy(out=out, in_=in_)


def _build():
    nc = bacc.Bacc(
        "TRN2", target_bir_lowering=False, debug=False, num_devices=NCORES
    )
    xt_ap = nc.dram_tensor("xt", [NK, 128, BSH], BF16, kind="ExternalInput").ap()
    bd_ap = nc.dram_tensor("bd", [NK, 128, 128], BF16, kind="ExternalInput").ap()
    kf_ap = nc.dram_tensor("kf", [128, 128], BF16, kind="ExternalInput").ap()
    zt_ap = nc.dram_tensor("zt", [NV, 128, BSH], BF16, kind="ExternalOutput").ap()

    with tile.TileContext(nc) as tc, ExitStack() as ctx:
        consts = ctx.enter_context(tc.tile_pool(name="consts", bufs=1))
        bd_sb = consts.tile([128, NK, 128], BF16)
        nc.sync.dma_start(bd_sb, bd_ap.rearrange("k p c -> p k c"))
        kf_sb = consts.tile([128, 128], BF16)
        nc.sync.dma_start(kf_sb, kf_ap)

        xpool = ctx.enter_context(tc.tile_pool(name="xt", bufs=2))
        ytpool = ctx.enter_context(tc.tile_pool(name="yt", bufs=2))
        yvpool = ctx.enter_context(tc.tile_pool(name="yv", bufs=3))
        ztpool = ctx.enter_context(tc.tile_pool(name="zt", bufs=3))
        ps1 = ctx.enter_context(tc.tile_pool(name="ps1", bufs=3, space="PSUM"))
        ps2 = ctx.enter_context(tc.tile_pool(name="ps2", bufs=3, space="PSUM"))

        xt_r = xt_ap.rearrange("k p b -> p k b")

        def load_x(sc):
            xk = xpool.tile([128, NK, SC], BF16, tag="x")
            nc.sync.dma_start(xk, xt_r[:, :, sc * SC:(sc + 1) * SC])
            return xk

        x_tiles = {0: load_x(0)}

        for sc in range(NSC):
            b0 = sc * SC
            xk = x_tiles.pop(sc)
            if sc + 1 < NSC:
                x_tiles[sc + 1] = load_x(sc + 1)

            # ---- step 1 per k-tile ----
            yt_tiles = []
            for k in range(NK):
                yt_k = ytpool.tile([128, SC], BF16, tag=f"yt{k}")
                for j in range(NJ):
                    yp = ps1.tile([128, MOV], F32)
                    nc.tensor.matmul(
                        yp,
                        bd_sb[:, k, :],
                        xk[:, k, j * MOV:(j + 1) * MOV],
                        start=True,
                        stop=True,
                    )
                    _evict_op(nc, k * NJ + j,
                              yt_k[:, j * MOV:(j + 1) * MOV], yp)
                yt_tiles.append(yt_k)

            # ---- deinterleave + step 2 + z evict ----
            for v in range(NV):
                zt_sb = ztpool.tile([128, SC], BF16, tag=f"z{v % 3}")
                yv_sb = yvpool.tile([128, SC], BF16, tag=f"yv{v % 3}")
                for k in range(NK):
                    src = yt_tiles[k].rearrange("(i v) b -> i v b", v=NV)
                    eng = nc.sync if k < 6 else nc.scalar
                    eng.dma_start(
                        yv_sb[16 * k:16 * (k + 1), :], src[:, v, :]
                    )
                for j in range(NJ):
                    zp = ps2.tile([128, MOV], F32)
                    nc.tensor.matmul(
                        zp,
                        kf_sb,
                        yv_sb[:, j * MOV:(j + 1) * MOV],
                        start=True,
                        stop=True,
                    )
                    _evict_op(nc, v * NJ + j + 4,
                              zt_sb[:, j * MOV:(j + 1) * MOV], zp)
                nc.sync.dma_start(zt_ap[v, :, b0:b0 + SC], zt_sb)

    nc.compile()
    return nc


def kernel(x, kernel_seq, kernel_full):
    global LAST_EXEC_NS
    x = np.asarray(x, dtype=np.float32)
    ks = np.asarray(kernel_seq, dtype=np.float32)
    kf = np.asarray(kernel_full, dtype=np.float32)

    # --- host-side weight prep ---
    bd = np.zeros((NK, 128, 128), dtype=np.float32)
    for k in range(NK):
        for i in range(16):
            bd[k, i * 8:(i + 1) * 8, i * 8:(i + 1) * 8] = ks[k * 16 + i]
    bd16 = bd.astype(ml_dtypes.bfloat16)
    kf16 = np.ascontiguousarray(kf).astype(ml_dtypes.bfloat16)

    # --- host-side input layout: per-core transposed channel-major bf16 ---
    x16 = x.astype(ml_dtypes.bfloat16)
    xt = np.ascontiguousarray(
        x16.reshape(NCORES, BSH, C).transpose(0, 2, 1)
    ).reshape(NCORES, NK, 128, BSH)

    if "nc" not in _cache:
        _cache["nc"] = _build()
    nc = _cache["nc"]

    in_maps = [
        {"xt": xt[i], "bd": bd16, "kf": kf16} for i in range(NCORES)
    ]
    kw = {}
    if TRACE:
        _setup_trace_shim()
        global LAST_TRACE_DIR
        import tempfile

        LAST_TRACE_DIR = tempfile.mkdtemp(prefix="ktrace_")
        kw = {"tmpdir": LAST_TRACE_DIR}
    res = run_bass_kernel_spmd(nc, in_maps, list(range(NCORES)), trace=TRACE, **kw)
    if res.exec_time_ns is not None:
        LAST_EXEC_NS = res.exec_time_ns

    # --- host-side output reassembly ---
    zt = np.stack([np.asarray(r["zt"]) for r in res.results], axis=0)
    z = zt.astype(np.float32).transpose(0, 3, 2, 1).reshape(B, C)
    return np.ascontiguousarray(z)


# revision 10
# speedup vs baseline: 1.0043x; 1.0043x over previous
"""Trainium2 Bass kernel for nn_GroupDenseFull.

Math: z[b, t*8+v] = sum_{s,w} x[b, s*8+w] * kernel_seq[s,w,v] * kernel_full[s,t]

Two-step structure (7.5x fewer FLOPs than the fused x @ Wc):
  step 1 (grouped):  y[b,s,v] = sum_w x[b,s,w] * ks[s,w,v]
  step 2 (mixing):   z[b,t,v] = sum_s y[b,s,v] * kf[s,t]

Device-side design:
  - bf16 I/O. Host pre-transposes x to channel-major [8k, 128c, B] bf16 so the
    device never transposes; host reassembles the transposed bf16 z output.
  - step 1 on PE: per k-tile of 128 channels ((s,w) interleaved, 16 groups),
    stationary = 128x128 block-diagonal ks matrix -> yT tiles with (s,v)
    interleaved partitions. Full PE utilization, 1 cyc/row (bf16, 512-mov).
  - deinterleave (s,v)-interleaved yT -> v-separated y_v via SBUF->SBUF DMA:
    per (v,k) one DMA of 16 partition lines (stride 8 -> contiguous), 4KB
    contiguous per line.
  - step 2 on PE: stationary = kf, moving = y_v [s, b] -> zT_v [t, b].
  - PSUM evict-casts (f32 -> bf16) spread across ACT, DVE, and GpSimd.
"""

import os
from contextlib import ExitStack

import numpy as np
import ml_dtypes

import concourse.bass as bass
import concourse.tile as tile
from concourse import bacc, mybir
from concourse.bass_utils import run_bass_kernel_spmd

B, C, W, S = 131072, 1024, 8, 128
NCORES = 8
BSH = B // NCORES          # 16384 rows per core
NK = 8                     # channel k-tiles of 128
NV = 8                     # v planes
SC = 2048                  # batch super-chunk (4KB bf16 DMA lines)
NSC = BSH // SC            # 8 super-chunks
MOV = 512                  # matmul moving width
NJ = SC // MOV             # 4 moving blocks per super-chunk

F32 = mybir.dt.float32
BF16 = mybir.dt.bfloat16

TRACE = bool(int(os.environ.get("KERNEL_TRACE", "0")))
LAST_EXEC_NS = None
LAST_TRACE_DIR = None

_cache = {}


def _setup_trace_shim():
    import sys
    import types

    import antenv
    from trn_agent_boot.trn_boot import _ntff_profile_via_ctypes

    if "antenv.axon_hooks" in sys.modules:
        return
    mod = types.ModuleType("antenv.axon_hooks")
    mod._hook = _ntff_profile_via_ctypes("/opt/axon/libaxon_pjrt.so")
    mod.get_axon_ntff_profile_hook = lambda: mod._hook
    mod.set_axon_ntff_profile_hook = lambda h: setattr(mod, "_hook", h)
    sys.modules["antenv.axon_hooks"] = mod
    antenv.axon_hooks = mod
    import concourse.bass_utils as bu

    bu.upload_artifacts = lambda tmpdir: tmpdir


def _noop():
    pass


def _build():
    nc = bacc.Bacc(
        "TRN2", target_bir_lowering=False, debug=False, num_devices=NCORES
    )
    xt_ap = nc.dram_tensor("xt", [NK, 128, BSH], BF16, kind="ExternalInput").ap()
    bd_ap = nc.dram_tensor("bd", [NK, 128, 128], BF16, kind="ExternalInput").ap()
    kf_ap = nc.dram_tensor("kf", [128, 128], BF16, kind="ExternalInput").ap()
    zt_ap = nc.dram_tensor("zt", [NV, 128, BSH], BF16, kind="ExternalOutput").ap()

    with tile.TileContext(nc) as tc, ExitStack() as ctx:
        consts = ctx.enter_context(tc.tile_pool(name="consts", bufs=1))
        bd_sb = consts.tile([128, NK, 128], BF16)
        nc.sync.dma_start(bd_sb, bd_ap.rearrange("k p c -> p k c"))
        kf_sb = consts.tile([128, 128], BF16)
        nc.sync.dma_start(kf_sb, kf_ap)

        xpool = ctx.enter_context(tc.tile_pool(name="xt", bufs=2))
        ytpool = ctx.enter_context(tc.tile_pool(name="yt", bufs=2))
        yvpool = ctx.enter_context(tc.tile_pool(name="yv", bufs=3))
        ztpool = ctx.enter_context(tc.tile_pool(name="zt", bufs=3))
        ps1 = ctx.enter_context(tc.tile_pool(name="ps1", bufs=2, space="PSUM"))
        ps2 = ctx.enter_context(tc.tile_pool(name="ps2", bufs=2, space="PSUM"))


        for sc in range(NSC):
            b0 = sc * SC

            # ---- x loads (ACT queue), step 1 per k-tile ----
            x_tiles = []
            for k in range(NK):
                xk = xpool.tile([128, SC], BF16, tag=f"x{k}")
                nc.scalar.dma_start(xk, xt_ap[k, :, b0:b0 + SC])
                x_tiles.append(xk)
            yt_tiles = []
            for k in range(NK):
                yt_k = ytpool.tile([128, SC], BF16, tag=f"yt{k}")
                for jj in range(NJ // 2):
                    yp = ps1.tile([128, 2 * MOV], F32)
                    for h in range(2):
                        nc.tensor.matmul(
                            yp[:, h * MOV:(h + 1) * MOV],
                            bd_sb[:, k, :],
                            x_tiles[k][:, (2 * jj + h) * MOV:(2 * jj + h + 1) * MOV],
                            start=True,
                            stop=True,
                        )
                    nc.scalar.copy(
                        out=yt_k[:, 2 * jj * MOV:(2 * jj + 2) * MOV], in_=yp
                    )
                yt_tiles.append(yt_k)

            # ---- deinterleave + step 2 + z evict ----
            for v in range(NV):
                if v % 2 == 0:
                    zt_sb = ztpool.tile([128, 2, SC], BF16, tag="z")
                yv_sb = yvpool.tile([128, SC], BF16, tag="yv")
                for k in range(NK):
                    src = yt_tiles[k].rearrange("(i v) b -> i v b", v=NV)
                    eng = nc.sync if k < 6 else nc.scalar
                    eng.dma_start(
                        yv_sb[16 * k:16 * (k + 1), :], src[:, v, :]
                    )
                for jj in range(NJ // 2):
                    zp = ps2.tile([128, 2 * MOV], F32)
                    for h in range(2):
                        nc.tensor.matmul(
                            zp[:, h * MOV:(h + 1) * MOV],
                            kf_sb,
                            yv_sb[:, (2 * jj + h) * MOV:(2 * jj + h + 1) * MOV],
                            start=True,
                            stop=True,
                        )
                    nc.vector.tensor_copy(
                        out=zt_sb[:, v % 2, 2 * jj * MOV:(2 * jj + 2) * MOV],
                        in_=zp,
                    )
                if v % 2 == 1:
                    nc.sync.dma_start(
                        zt_ap[v - 1:v + 1, :, b0:b0 + SC].rearrange(
                            "v t b -> t v b"
                        ),
                        zt_sb,
                    )

    nc.compile()
    return nc


def kernel(x, kernel_seq, kernel_full):
    global LAST_EXEC_NS
    x = np.asarray(x, dtype=np.float32)
    ks = np.asarray(kernel_seq, dtype=np.float32)
    kf = np.asarray(kernel_full, dtype=np.float32)

    # --- host-side weight prep ---
    bd = np.zeros((NK, 128, 128), dtype=np.float32)
    for k in range(NK):
        for i in range(16):
            bd[k, i * 8:(i + 1) * 8, i * 8:(i + 1) * 8] = ks[k * 16 + i]
    bd16 = bd.astype(ml_dtypes.bfloat16)
    kf16 = np.ascontiguousarray(kf).astype(ml_dtypes.bfloat16)

    # --- host-side input layout: per-core transposed channel-major bf16 ---
    x16 = x.astype(ml_dtypes.bfloat16)
    xt = np.ascontiguousarray(
        x16.reshape(NCORES, BSH, C).transpose(0, 2, 1)
    ).reshape(NCORES, NK, 128, BSH)

    if "nc" not in _cache:
        _cache["nc"] = _build()
    nc = _cache["nc"]

    in_maps = [
        {"xt": xt[i], "bd": bd16, "kf": kf16} for i in range(NCORES)
    ]
    kw = {}
    if TRACE:
        _setup_trace_shim()
        global LAST_TRACE_DIR
        import tempfile

        LAST_TRACE_DIR = tempfile.mkdtemp(prefix="ktrace_")
        kw = {"tmpdir": LAST_TRACE_DIR}
    res = run_bass_kernel_spmd(nc, in_maps, list(range(NCORES)), trace=TRACE, **kw)
    if res.exec_time_ns is not None:
        LAST_EXEC_NS = res.exec_time_ns

    # --- host-side output reassembly ---
    zt = np.stack([np.asarray(r["zt"]) for r in res.results], axis=0)
    z = zt.astype(np.float32).transpose(0, 3, 2, 1).reshape(B, C)
    return np.ascontiguousarray(z)


# revision 12
# speedup vs baseline: 1.5239x; 1.5174x over previous
"""Trainium2 Bass kernel for nn_GroupDenseFull.

Math: z[b, t*8+v] = sum_{s,w} x[b, s*8+w] * kernel_seq[s,w,v] * kernel_full[s,t]

Two-step structure (7.5x fewer FLOPs than the fused x @ Wc):
  step 1 (grouped):  y[b,s,v] = sum_w x[b,s,w] * ks[s,w,v]
  step 2 (mixing):   z[b,t,v] = sum_s y[b,s,v] * kf[s,t]

Device-side design:
  - bf16 I/O. Host pre-transposes x to channel-major [8k, 128c, B] bf16 so the
    device never transposes; host reassembles the transposed bf16 z output.
  - step 1 on PE: per k-tile of 128 channels ((s,w) interleaved, 16 groups),
    stationary = 128x128 block-diagonal ks matrix -> yT tiles with (s,v)
    interleaved partitions. Full PE utilization, 1 cyc/row (bf16, 512-mov).
  - deinterleave (s,v)-interleaved yT -> v-separated y_v via SBUF->SBUF DMA:
    per (v,k) one DMA of 16 partition lines (stride 8 -> contiguous), 4KB
    contiguous per line.
  - step 2 on PE: stationary = kf, moving = y_v [s, b] -> zT_v [t, b].
  - PSUM evict-casts (f32 -> bf16) spread across ACT, DVE, and GpSimd.
"""

import os
from contextlib import ExitStack

import numpy as np
import ml_dtypes

import concourse.bass as bass
import concourse.tile as tile
from concourse import bacc, mybir
from concourse.bass_utils import run_bass_kernel_spmd

B, C, W, S = 131072, 1024, 8, 128
NCORES = 8
BSH = B // NCORES          # 16384 rows per core
NK = 8                     # channel k-tiles of 128
NV = 8                     # v planes
SC = 2048                  # batch super-chunk (4KB bf16 DMA lines)
NSC = BSH // SC            # 8 super-chunks
MOV = 512                  # matmul moving width
NJ = SC // MOV             # 4 moving blocks per super-chunk

F32 = mybir.dt.float32
BF16 = mybir.dt.bfloat16

TRACE = bool(int(os.environ.get("KERNEL_TRACE", "0")))
LAST_EXEC_NS = None
LAST_TRACE_DIR = None

_cache = {}


def _setup_trace_shim():
    import sys
    import types

    import antenv
    from trn_agent_boot.trn_boot import _ntff_profile_via_ctypes

    if "antenv.axon_hooks" in sys.modules:
        return
    mod = types.ModuleType("antenv.axon_hooks")
    mod._hook = _ntff_profile_via_ctypes("/opt/axon/libaxon_pjrt.so")
    mod.get_axon_ntff_profile_hook = lambda: mod._hook
    mod.set_axon_ntff_profile_hook = lambda h: setattr(mod, "_hook", h)
    sys.modules["antenv.axon_hooks"] = mod
    antenv.axon_hooks = mod
    import concourse.bass_utils as bu

    bu.upload_artifacts = lambda tmpdir: tmpdir


def _noop():
    pass


def _build():
    nc = bacc.Bacc(
        "TRN2", target_bir_lowering=False, debug=False, num_devices=NCORES
    )
    xt_ap = nc.dram_tensor("xt", [NK, 128, BSH], BF16, kind="ExternalInput").ap()
    bd_ap = nc.dram_tensor("bd", [NK, 128, 128], BF16, kind="ExternalInput").ap()
    kf_ap = nc.dram_tensor("kf", [128, 128], BF16, kind="ExternalInput").ap()
    zt_ap = nc.dram_tensor("zt", [NV, 128, BSH], BF16, kind="ExternalOutput").ap()

    with tile.TileContext(nc) as tc, ExitStack() as ctx:
        consts = ctx.enter_context(tc.tile_pool(name="consts", bufs=1))
        bd_sb = consts.tile([128, NK, 128], BF16)
        nc.sync.dma_start(bd_sb, bd_ap.rearrange("k p c -> p k c"))
        kf_sb = consts.tile([128, 128], BF16)
        nc.sync.dma_start(kf_sb, kf_ap)

        xpool = ctx.enter_context(tc.tile_pool(name="xt", bufs=2))
        ytpool = ctx.enter_context(tc.tile_pool(name="yt", bufs=2))
        yvpool = ctx.enter_context(tc.tile_pool(name="yv", bufs=3))
        ztpool = ctx.enter_context(tc.tile_pool(name="zt", bufs=3))
        ps1 = ctx.enter_context(tc.tile_pool(name="ps1", bufs=2, space="PSUM"))
        ps2 = ctx.enter_context(tc.tile_pool(name="ps2", bufs=2, space="PSUM"))


        for sc in range(NSC):
            b0 = sc * SC

            # ---- x loads (ACT queue), step 1 per k-tile ----
            x_tiles = []
            for k in range(NK):
                xk = xpool.tile([128, SC], BF16, tag=f"x{k}")
                nc.scalar.dma_start(xk, xt_ap[k, :, b0:b0 + SC])
                x_tiles.append(xk)
            yt_sb = ytpool.tile([128, NK, SC], BF16, tag="yt")
            for k in range(NK):
                for jj in range(NJ // 2):
                    yp = ps1.tile([128, 2 * MOV], F32)
                    for h in range(2):
                        nc.tensor.matmul(
                            yp[:, h * MOV:(h + 1) * MOV],
                            bd_sb[:, k, :],
                            x_tiles[k][:, (2 * jj + h) * MOV:(2 * jj + h + 1) * MOV],
                            start=True,
                            stop=True,
                        )
                    nc.scalar.copy(
                        out=yt_sb[:, k, 2 * jj * MOV:(2 * jj + 2) * MOV], in_=yp
                    )

            # ---- deinterleave + step 2 + z evict ----
            for v in range(NV):
                if v % 2 == 0:
                    zt_sb = ztpool.tile([128, 2, SC], BF16, tag="z")
                yv_sb = yvpool.tile([128, SC], BF16, tag="yv")
                # src: partitions (i*8+v) stride 8 (leading), free (k, b)
                # dst: partitions p' = 8i + k contiguous 0..127; s-order
                # permutation folded into kf rows on the host.
                nc.sync.dma_start(
                    yv_sb,
                    yt_sb.rearrange("(i v) k b -> i v k b", v=NV)[:, v, :, :],
                )
                for jj in range(NJ // 2):
                    zp = ps2.tile([128, 2 * MOV], F32)
                    for h in range(2):
                        nc.tensor.matmul(
                            zp[:, h * MOV:(h + 1) * MOV],
                            kf_sb,
                            yv_sb[:, (2 * jj + h) * MOV:(2 * jj + h + 1) * MOV],
                            start=True,
                            stop=True,
                        )
                    nc.vector.tensor_copy(
                        out=zt_sb[:, v % 2, 2 * jj * MOV:(2 * jj + 2) * MOV],
                        in_=zp,
                    )
                if v % 2 == 1:
                    nc.sync.dma_start(
                        zt_ap[v - 1:v + 1, :, b0:b0 + SC].rearrange(
                            "v t b -> t v b"
                        ),
                        zt_sb,
                    )

    nc.compile()
    return nc


def kernel(x, kernel_seq, kernel_full):
    global LAST_EXEC_NS
    x = np.asarray(x, dtype=np.float32)
    ks = np.asarray(kernel_seq, dtype=np.float32)
    kf = np.asarray(kernel_full, dtype=np.float32)

    # --- host-side weight prep ---
    bd = np.zeros((NK, 128, 128), dtype=np.float32)
    for k in range(NK):
        for i in range(16):
            bd[k, i * 8:(i + 1) * 8, i * 8:(i + 1) * 8] = ks[k * 16 + i]
    bd16 = bd.astype(ml_dtypes.bfloat16)
    perm = np.array([16 * (p % 8) + p // 8 for p in range(128)])
    kf16 = np.ascontiguousarray(kf[perm, :]).astype(ml_dtypes.bfloat16)

    # --- host-side input layout: per-core transposed channel-major bf16 ---
    x16 = x.astype(ml_dtypes.bfloat16)
    xt = np.ascontiguousarray(
        x16.reshape(NCORES, BSH, C).transpose(0, 2, 1)
    ).reshape(NCORES, NK, 128, BSH)

    if "nc" not in _cache:
        _cache["nc"] = _build()
    nc = _cache["nc"]

    in_maps = [
        {"xt": xt[i], "bd": bd16, "kf": kf16} for i in range(NCORES)
    ]
    kw = {}
    if TRACE:
        _setup_trace_shim()
        global LAST_TRACE_DIR
        import tempfile

        LAST_TRACE_DIR = tempfile.mkdtemp(prefix="ktrace_")
        kw = {"tmpdir": LAST_TRACE_DIR}
    res = run_bass_kernel_spmd(nc, in_maps, list(range(NCORES)), trace=TRACE, **kw)
    if res.exec_time_ns is not None:
        LAST_EXEC_NS = res.exec_time_ns

    # --- host-side output reassembly ---
    zt = np.stack([np.asarray(r["zt"]) for r in res.results], axis=0)
    z = zt.astype(np.float32).transpose(0, 3, 2, 1).reshape(B, C)
    return np.ascontiguousarray(z)


# revision 13
# speedup vs baseline: 1.6479x; 1.0814x over previous
"""Trainium2 Bass kernel for nn_GroupDenseFull.

Math: z[b, t*8+v] = sum_{s,w} x[b, s*8+w] * kernel_seq[s,w,v] * kernel_full[s,t]

Two-step structure (7.5x fewer FLOPs than the fused x @ Wc):
  step 1 (grouped):  y[b,s,v] = sum_w x[b,s,w] * ks[s,w,v]
  step 2 (mixing):   z[b,t,v] = sum_s y[b,s,v] * kf[s,t]

Device-side design:
  - bf16 I/O. Host pre-transposes x to channel-major [8k, 128c, B] bf16 so the
    device never transposes; host reassembles the transposed bf16 z output.
  - step 1 on PE: per k-tile of 128 channels ((s,w) interleaved, 16 groups),
    stationary = 128x128 block-diagonal ks matrix -> yT tiles with (s,v)
    interleaved partitions. Full PE utilization, 1 cyc/row (bf16, 512-mov).
  - deinterleave (s,v)-interleaved yT -> v-separated y_v via SBUF->SBUF DMA:
    per (v,k) one DMA of 16 partition lines (stride 8 -> contiguous), 4KB
    contiguous per line.
  - step 2 on PE: stationary = kf, moving = y_v [s, b] -> zT_v [t, b].
  - PSUM evict-casts (f32 -> bf16) spread across ACT, DVE, and GpSimd.
"""

import os
from contextlib import ExitStack

import numpy as np
import ml_dtypes

import concourse.bass as bass
import concourse.tile as tile
from concourse import bacc, mybir
from concourse.bass_utils import run_bass_kernel_spmd

B, C, W, S = 131072, 1024, 8, 128
NCORES = 8
BSH = B // NCORES          # 16384 rows per core
NK = 8                     # channel k-tiles of 128
NV = 8                     # v planes
SC = 2048                  # batch super-chunk (4KB bf16 DMA lines)
NSC = BSH // SC            # 8 super-chunks
MOV = 512                  # matmul moving width
NJ = SC // MOV             # 4 moving blocks per super-chunk

F32 = mybir.dt.float32
BF16 = mybir.dt.bfloat16

TRACE = bool(int(os.environ.get("KERNEL_TRACE", "0")))
LAST_EXEC_NS = None
LAST_TRACE_DIR = None

_cache = {}


def _setup_trace_shim():
    import sys
    import types

    import antenv
    from trn_agent_boot.trn_boot import _ntff_profile_via_ctypes

    if "antenv.axon_hooks" in sys.modules:
        return
    mod = types.ModuleType("antenv.axon_hooks")
    mod._hook = _ntff_profile_via_ctypes("/opt/axon/libaxon_pjrt.so")
    mod.get_axon_ntff_profile_hook = lambda: mod._hook
    mod.set_axon_ntff_profile_hook = lambda h: setattr(mod, "_hook", h)
    sys.modules["antenv.axon_hooks"] = mod
    antenv.axon_hooks = mod
    import concourse.bass_utils as bu

    bu.upload_artifacts = lambda tmpdir: tmpdir


def _noop():
    pass


def _build():
    nc = bacc.Bacc(
        "TRN2", target_bir_lowering=False, debug=False, num_devices=NCORES
    )
    xt_ap = nc.dram_tensor("xt", [NK, 128, BSH], BF16, kind="ExternalInput").ap()
    bd_ap = nc.dram_tensor("bd", [NK, 128, 128], BF16, kind="ExternalInput").ap()
    kf_ap = nc.dram_tensor("kf", [128, 128], BF16, kind="ExternalInput").ap()
    zt_ap = nc.dram_tensor("zt", [NV, 128, BSH], BF16, kind="ExternalOutput").ap()

    with tile.TileContext(nc) as tc, ExitStack() as ctx:
        consts = ctx.enter_context(tc.tile_pool(name="consts", bufs=1))
        bd_sb = consts.tile([128, NK, 128], BF16)
        nc.sync.dma_start(bd_sb, bd_ap.rearrange("k p c -> p k c"))
        kf_sb = consts.tile([128, 128], BF16)
        nc.sync.dma_start(kf_sb, kf_ap)

        xpool = ctx.enter_context(tc.tile_pool(name="xt", bufs=2))
        ytpool = ctx.enter_context(tc.tile_pool(name="yt", bufs=2))
        yvpool = ctx.enter_context(tc.tile_pool(name="yv", bufs=3))
        ztpool = ctx.enter_context(tc.tile_pool(name="zt", bufs=3))
        ps1 = ctx.enter_context(tc.tile_pool(name="ps1", bufs=2, space="PSUM"))
        ps2 = ctx.enter_context(tc.tile_pool(name="ps2", bufs=2, space="PSUM"))


        for sc in range(NSC):
            b0 = sc * SC

            # ---- x loads (ACT queue), step 1 per k-tile ----
            x_tiles = []
            for k in range(NK):
                xk = xpool.tile([128, SC], BF16, tag=f"x{k}")
                nc.scalar.dma_start(xk, xt_ap[k, :, b0:b0 + SC])
                x_tiles.append(xk)
            yt_sb = ytpool.tile([128, NK, SC], BF16, tag="yt")
            for k in range(NK):
                for jj in range(NJ // 2):
                    yp = ps1.tile([128, 2 * MOV], F32)
                    for h in range(2):
                        nc.tensor.matmul(
                            yp[:, h * MOV:(h + 1) * MOV],
                            bd_sb[:, k, :],
                            x_tiles[k][:, (2 * jj + h) * MOV:(2 * jj + h + 1) * MOV],
                            start=True,
                            stop=True,
                        )
                    nc.scalar.copy(
                        out=yt_sb[:, k, 2 * jj * MOV:(2 * jj + 2) * MOV], in_=yp
                    )

            # ---- deinterleave (all v up-front), then step 2 + z evict ----
            yv_tiles = []
            for v in range(NV):
                yv_sb = yvpool.tile([128, SC], BF16, tag=f"yv{v % 2}")
                # src: partitions (i*8+v) stride 8 (leading), free (k, b)
                # dst: partitions p' = 8i + k contiguous 0..127; s-order
                # permutation folded into kf rows on the host.
                nc.sync.dma_start(
                    yv_sb,
                    yt_sb.rearrange("(i v) k b -> i v k b", v=NV)[:, v, :, :],
                )
                yv_tiles.append(yv_sb)
            for v in range(NV):
                if v % 2 == 0:
                    zt_sb = ztpool.tile([128, 2, SC], BF16, tag="z")
                yv_sb = yv_tiles[v]
                for jj in range(NJ // 2):
                    zp = ps2.tile([128, 2 * MOV], F32)
                    for h in range(2):
                        nc.tensor.matmul(
                            zp[:, h * MOV:(h + 1) * MOV],
                            kf_sb,
                            yv_sb[:, (2 * jj + h) * MOV:(2 * jj + h + 1) * MOV],
                            start=True,
                            stop=True,
                        )
                    nc.vector.tensor_copy(
                        out=zt_sb[:, v % 2, 2 * jj * MOV:(2 * jj + 2) * MOV],
                        in_=zp,
                    )
                if v % 2 == 1:
                    nc.sync.dma_start(
                        zt_ap[v - 1:v + 1, :, b0:b0 + SC].rearrange(
                            "v t b -> t v b"
                        ),
                        zt_sb,
                    )

    nc.compile()
    return nc


def kernel(x, kernel_seq, kernel_full):
    global LAST_EXEC_NS
    x = np.asarray(x, dtype=np.float32)
    ks = np.asarray(kernel_seq, dtype=np.float32)
    kf = np.asarray(kernel_full, dtype=np.float32)

    # --- host-side weight prep ---
    bd = np.zeros((NK, 128, 128), dtype=np.float32)
    for k in range(NK):
        for i in range(16):
            bd[k, i * 8:(i + 1) * 8, i * 8:(i + 1) * 8] = ks[k * 16 + i]
    bd16 = bd.astype(ml_dtypes.bfloat16)
    perm = np.array([16 * (p % 8) + p // 8 for p in range(128)])
    kf16 = np.ascontiguousarray(kf[perm, :]).astype(ml_dtypes.bfloat16)

    # --- host-side input layout: per-core transposed channel-major bf16 ---
    x16 = x.astype(ml_dtypes.bfloat16)
    xt = np.ascontiguousarray(
        x16.reshape(NCORES, BSH, C).transpose(0, 2, 1)
    ).reshape(NCORES, NK, 128, BSH)

    if "nc" not in _cache:
        _cache["nc"] = _build()
    nc = _cache["nc"]

    in_maps = [
        {"xt": xt[i], "bd": bd16, "kf": kf16} for i in range(NCORES)
    ]
    kw = {}
    if TRACE:
        _setup_trace_shim()
        global LAST_TRACE_DIR
        import tempfile

        LAST_TRACE_DIR = tempfile.mkdtemp(prefix="ktrace_")
        kw = {"tmpdir": LAST_TRACE_DIR}
    res = run_bass_kernel_spmd(nc, in_maps, list(range(NCORES)), trace=TRACE, **kw)
    if res.exec_time_ns is not None:
        LAST_EXEC_NS = res.exec_time_ns

    # --- host-side output reassembly ---
    zt = np.stack([np.asarray(r["zt"]) for r in res.results], axis=0)
    z = zt.astype(np.float32).transpose(0, 3, 2, 1).reshape(B, C)
    return np.ascontiguousarray(z)
